# revision 14
# baseline (speedup 1.0000x reference)
"""Trainium2 Bass kernel for DissimilarityMixtureEncoderCov forward.

Computes softmax(-ALPHA * D + log(relu(mixers)), axis=-1) where
  D[b,k] = (x_b - mu_k)^T (C_k C_k^T) (x_b - mu_k)

Data-parallel over batch across 8 NeuronCores. Per core, using
D = ||C^T x - C^T mu||^2 expanded as T1 - 2a x.v + const with
Y[b,(k,j)] = x_b . C_k[:,j] computed in "2.5 passes" on the PE:

  main:  Xr.Cr        fp32r matmul (1.0 cycle/row)
  DR1:   Ex.(c1+c2)   fp8 DoubleRow (0.5): Ex in e4m3@2^10 (both pair
                      slots), c-side e5m2 2-term expansion of C @2^-10
  DR2:   (x1+x2).Ec   fp8 DoubleRow: x-side e5m2 2-term @2^-10,
                      Ec in e4m3@2^10 (both slots)
  DR3:   Ex2.c1 + X.Ec2   fp8 DoubleRow, all e5m2 @2^±14: second-order
                      residuals of the e4m3 quantizations above

All splits/quantizations are computed on the host in numpy and shipped
as pre-rounded fp32r / fp8 dram tensors (the BIR verifier requires
fp32r matmul inputs to be produced pre-rounded, which host data is).
x ships pre-transposed; no on-device transposes or conversions at all.

  T1[b,k] = ALPHA * sum_j Y^2  -- ACT square slab (PSUM->SBUF) + DVE
      grouped reduce per (block, chunk); a few groups per chunk use the
      fused ACT square+accum path to balance ACT vs DVE.
  t/v path on a [d,(g,j)] full-cov stream: t_bc = Pool
      partition_all_reduce of cov*mu2a (result broadcast to all
      partitions), v^T = Pool mult + DVE grouped reduce -> vt2a
      directly in [d,k] layout.  const = -a||t||^2 + ln(mix) recovered
      from mu.v via a Pool partition reduce (||t||^2 = mu.v).
  logits+softmax epilogue after the last block.
"""

import sys

sys.path.insert(0, "/opt/trn_rl_repo")

import numpy as np

import concourse.bacc as bacc
import concourse.tile as tile
from concourse import mybir

ALPHA = 10.0
B, K, D = 8192, 128, 128
N_CORES = 8
B_LOC = B // N_CORES          # 1024 batch rows per core
N_CHUNKS = B_LOC // 128       # 8 chunks of 128 rows
KJ = K * D                    # 16384 columns of the big matmul
BLK = 2048                    # psum block = 4 banks; 16 k-groups
N_BLK = KJ // BLK             # 8 blocks
NGRP = BLK // 128             # 16 k-groups per block
SQRT_A = float(np.sqrt(ALPHA))

# per-chunk fused-group count (ACT square+accum); rest reduced on DVE
NF_CHUNK = [1, 0, 0, 0, 1, 0, 0, 0]

FP32 = mybir.dt.float32
FP32R = mybir.dt.float32r
E4 = mybir.dt.float8e4
E5 = mybir.dt.float8e5


def _build_bass():
    nc = bacc.Bacc("TRN2", target_bir_lowering=False, debug=False,
                   num_devices=N_CORES)

    PM = mybir.MatmulPerfMode

    xtr_d = nc.dram_tensor("xtr", [D, B_LOC], FP32R, kind="ExternalInput")
    xtf_d = nc.dram_tensor("xtf", [D, B_LOC], FP32, kind="ExternalInput")
    w1_d = nc.dram_tensor("w1", [D, 2 * B_LOC], E4, kind="ExternalInput")
    w2_d = nc.dram_tensor("w2", [D, 2 * B_LOC], E5, kind="ExternalInput")
    w3_d = nc.dram_tensor("w3", [D, 2 * B_LOC], E5, kind="ExternalInput")
    covr_d = nc.dram_tensor("covr", [K * D, D], FP32R, kind="ExternalInput")
    covf_d = nc.dram_tensor("covf", [K * D, D], FP32, kind="ExternalInput")
    covt_d = nc.dram_tensor("covt", [D, K * D], FP32, kind="ExternalInput")
    tscr_d = nc.dram_tensor("tscr", [N_BLK, BLK], FP32, kind="Internal")
    vscr_d = nc.dram_tensor("vscr", [N_BLK, BLK], FP32, kind="Internal")
    m1_d = nc.dram_tensor("m1", [D, N_BLK * 2 * BLK], E5, kind="ExternalInput")
    m2_d = nc.dram_tensor("m2", [D, N_BLK * 2 * BLK], E4, kind="ExternalInput")
    m3_d = nc.dram_tensor("m3", [D, N_BLK * 2 * BLK], E5, kind="ExternalInput")
    cent2a_d = nc.dram_tensor("cent2a", [D, K], FP32, kind="ExternalInput")
    cent_d = nc.dram_tensor("cent", [D, K], FP32, kind="ExternalInput")
    mix_d = nc.dram_tensor("mixers", [1, K], FP32, kind="ExternalInput")
    out_d = nc.dram_tensor("out", [B_LOC, K], FP32, kind="ExternalOutput")

    AF = mybir.ActivationFunctionType
    OP = mybir.AluOpType
    AX = mybir.AxisListType
    from concourse import bass_isa
    RED = bass_isa.ReduceOp

    with tile.TileContext(nc) as tc:
        with (
            tc.tile_pool(name="const", bufs=1) as constp,
            tc.tile_pool(name="covr", bufs=2) as covrp,
            tc.tile_pool(name="covf", bufs=2) as covfp,
            tc.tile_pool(name="covt", bufs=2) as covtp,
            tc.tile_pool(name="tt", bufs=2) as ttp,
            tc.tile_pool(name="var", bufs=2) as varp,
            tc.tile_pool(name="m1", bufs=2) as m1p,
            tc.tile_pool(name="m2", bufs=2) as m2p,
            tc.tile_pool(name="m3", bufs=2) as m3p,
            tc.tile_pool(name="prod", bufs=4) as prodp,
            tc.tile_pool(name="tbc", bufs=2) as tbcp,
            tc.tile_pool(name="xt", bufs=1) as xtp,
            tc.tile_pool(name="small", bufs=1) as smallp,
            tc.tile_pool(name="work", bufs=4) as workp,
            tc.tile_pool(name="t1a", bufs=1) as t1ap,
            tc.tile_pool(name="ysq", bufs=4) as ysqp,
            tc.tile_pool(name="py", bufs=2, space="PSUM") as pyp,
        ):
            def prep_block(blk, halves=1):
                c0 = blk * BLK
                hw_ = BLK // halves
                covr = covrp.tile([128, BLK], FP32R, tag="covr")
                for h in range(halves):
                    sl = slice(h * hw_, (h + 1) * hw_)
                    nc.sync.dma_start(
                        out=covr[:, sl].rearrange("d (g j) -> d g j", j=128),
                        in_=covr_d[c0 + h * hw_:c0 + (h + 1) * hw_, :]
                        .rearrange("(g d) j -> d g j", d=128),
                    )
                m1 = m1p.tile([128, 2 * BLK], E5, tag="m1")
                nc.sync.dma_start(
                    out=m1[:, :], in_=m1_d[:, blk * 2 * BLK:(blk + 1) * 2 * BLK])
                m2 = m2p.tile([128, 2 * BLK], E4, tag="m2")
                nc.sync.dma_start(
                    out=m2[:, :], in_=m2_d[:, blk * 2 * BLK:(blk + 1) * 2 * BLK])
                m3 = m3p.tile([128, 2 * BLK], E5, tag="m3")
                nc.sync.dma_start(
                    out=m3[:, :], in_=m3_d[:, blk * 2 * BLK:(blk + 1) * 2 * BLK])
                covf = covfp.tile([128, BLK], FP32, tag="covf")
                nc.sync.dma_start(
                    out=covf[:, :].rearrange("d (g j) -> d g j", j=128),
                    in_=covf_d[c0:c0 + BLK, :]
                    .rearrange("(g d) j -> d g j", d=128),
                )
                covt = covtp.tile([128, BLK], FP32, tag="covt")
                nc.sync.dma_start(out=covt[:, :],
                                  in_=covt_d[:, c0:c0 + BLK])
                return covr, m1, m2, m3, covf, covt

            # ---------- startup-critical DMAs in priority order ----------
            xtr = xtp.tile([128, B_LOC], FP32R)             # [d, b] hi
            nc.sync.dma_start(out=xtr[:, 0:128], in_=xtr_d[:, 0:128])
            covr0 = covrp.tile([128, BLK], FP32R, tag="covr")
            for q in range(2):
                nc.sync.dma_start(
                    out=covr0[:, q * 512:(q + 1) * 512].rearrange(
                        "d (g j) -> d g j", j=128),
                    in_=covr_d[q * 512:(q + 1) * 512, :].rearrange(
                        "(g d) j -> d g j", d=128))
            nc.sync.dma_start(out=xtr[:, 128:384], in_=xtr_d[:, 128:384])
            for q in range(2, 4):
                nc.sync.dma_start(
                    out=covr0[:, q * 512:(q + 1) * 512].rearrange(
                        "d (g j) -> d g j", j=128),
                    in_=covr_d[q * 512:(q + 1) * 512, :].rearrange(
                        "(g d) j -> d g j", d=128))
            # PE warmup: ramp the p-state while DMAs land
            warm = xtp.tile([128, 512], FP32R)
            nc.vector.memset(warm[:, :].bitcast(FP32), 0.0)
            wpy = pyp.tile([128, 512], FP32, tag="py")
            for _ in range(9):
                nc.tensor.matmul(wpy[:, :], warm[:, 0:128], warm[:, :],
                                 start=True, stop=True,
                                 skip_group_check=True)
            w1 = xtp.tile([128, 2 * B_LOC], E4)
            nc.sync.dma_start(out=w1[:, :], in_=w1_d[:, :])
            m1_0 = m1p.tile([128, 2 * BLK], E5, tag="m1")
            nc.sync.dma_start(out=m1_0[:, :], in_=m1_d[:, 0:2 * BLK])
            nc.sync.dma_start(out=xtr[:, 384:], in_=xtr_d[:, 384:])
            w2 = xtp.tile([128, 2 * B_LOC], E5)
            nc.sync.dma_start(out=w2[:, :], in_=w2_d[:, :])
            m2_0 = m2p.tile([128, 2 * BLK], E4, tag="m2")
            nc.sync.dma_start(out=m2_0[:, :], in_=m2_d[:, 0:2 * BLK])
            w3 = xtp.tile([128, 2 * B_LOC], E5)
            nc.sync.dma_start(out=w3[:, :], in_=w3_d[:, :])
            m3_0 = m3p.tile([128, 2 * BLK], E5, tag="m3")
            nc.sync.dma_start(out=m3_0[:, :], in_=m3_d[:, 0:2 * BLK])
            covf0 = covfp.tile([128, BLK], FP32, tag="covf")
            nc.sync.dma_start(
                out=covf0[:, :].rearrange("d (g j) -> d g j", j=128),
                in_=covf_d[0:BLK, :].rearrange("(g d) j -> d g j", d=128))
            covt0 = covtp.tile([128, BLK], FP32, tag="covt")
            nc.sync.dma_start(out=covt0[:, :], in_=covt_d[:, 0:BLK])
            prep0 = (covr0, m1_0, m2_0, m3_0, covf0, covt0)
            xtf = xtp.tile([128, B_LOC], FP32)              # [d, b] full
            nc.sync.dma_start(out=xtf[:, :], in_=xtf_d[:, :])
            cent2a = smallp.tile([128, K], FP32)   # [d, k] = 2a*mu^T
            nc.sync.dma_start(out=cent2a[:, :], in_=cent2a_d[:, :])
            cent = smallp.tile([128, K], FP32)     # [d, k] = mu^T
            nc.sync.dma_start(out=cent[:, :], in_=cent_d[:, :])
            mix = smallp.tile([1, K], FP32)
            nc.sync.dma_start(out=mix[:, :], in_=mix_d[:, :])

            # small helpers
            ones_row = constp.tile([1, 128], FP32)
            nc.vector.memset(ones_row[:, :], 1.0)
            bias_row = smallp.tile([1, K], FP32)
            nc.vector.tensor_scalar_max(bias_row[:, :], mix[:, :], 0.0)
            nc.scalar.activation(bias_row[:, :], bias_row[:, :], AF.Ln)

            vt2a_sb = smallp.tile([128, 128], FP32)  # [d, k] = 2a*v^T
            const_row = smallp.tile([1, K], FP32)

            # ---------- t/v chain on the covf [d,(g,j)] stream ----------
            tbc_cur = {}

            def t_mult(blk, covf):
                prod = prodp.tile([128, BLK], FP32, tag="prod")
                nc.gpsimd.tensor_tensor(
                    out=prod[:, :].rearrange("d (g j) -> d g j", j=128),
                    in0=covf[:, :].rearrange("d (g j) -> d g j", j=128),
                    in1=cent2a[:, blk * NGRP:(blk + 1) * NGRP].rearrange(
                        "d (g o) -> d g o", o=1).broadcast_to([128, NGRP, 128]),
                    op=OP.mult)
                tbc_cur['prod'] = prod

            def t_allred(blk):
                tbc = tbcp.tile([128, BLK], FP32, tag="tbc")
                nc.gpsimd.partition_all_reduce(
                    tbc[:, :], tbc_cur['prod'][:, :], channels=128,
                    reduce_op=RED.add)
                # roundtrip through dram to transpose t row -> [j, g]
                nc.sync.dma_start(out=tscr_d[blk, :], in_=tbc[0:1, :])
                tt = ttp.tile([128, NGRP], FP32, tag="tt")
                nc.sync.dma_start(
                    out=tt[:, :],
                    in_=tscr_d[blk, :].rearrange("(g j) -> j g", j=128))
                tbc_cur['tt'] = tt

            def v_mult(blk, covt):
                prod = prodp.tile([128, BLK], FP32, tag="prod")
                tt = tbc_cur['tt']
                nc.gpsimd.tensor_tensor(
                    out=prod[:, :].rearrange("j (g d) -> j g d", d=128),
                    in0=covt[:, :].rearrange("j (g d) -> j g d", d=128),
                    in1=tt[:, :].rearrange("j (g o) -> j g o", o=1)
                    .broadcast_to([128, NGRP, 128]),
                    op=OP.mult)
                tbc_cur['vprod'] = prod

            def v_red(blk):
                var = varp.tile([128, BLK], FP32, tag="var")
                nc.gpsimd.partition_all_reduce(
                    var[:, :], tbc_cur['vprod'][:, :], channels=128,
                    reduce_op=RED.add)
                nc.sync.dma_start(out=vscr_d[blk, :], in_=var[0:1, :])
                nc.sync.dma_start(
                    out=vt2a_sb[:, blk * NGRP:(blk + 1) * NGRP],
                    in_=vscr_d[blk, :].rearrange("(g d) -> d g", d=128))

            def const_chain():
                # ||t_k||^2 = mu_k . v_k ; vt2a holds 2a*v^T
                prod = workp.tile([128, 128], FP32, tag="cprod")
                nc.gpsimd.tensor_tensor(out=prod[:, :], in0=vt2a_sb[:, :],
                                        in1=cent[:, :], op=OP.mult)
                tnorm = workp.tile([1, 128], FP32, tag="tnorm")
                nc.gpsimd.tensor_reduce(out=tnorm[:, :], in_=prod[:, :],
                                        axis=AX.C, op=OP.add)
                # const = -a*||t||^2 + ln(mix) = -(2a mu.v)/2 + ln(mix)
                nc.scalar.activation(const_row[:, :], tnorm[:, :], AF.Copy,
                                     scale=-0.5)
                nc.vector.tensor_tensor(out=const_row[:, :],
                                        in0=const_row[:, :],
                                        in1=bias_row[:, :], op=OP.add)

            # slice schedule: (blk, chunk) -> action on block blk's t/v
            ACTIONS = {}
            for b in range(N_BLK - 1):
                ACTIONS[(b, 1)] = ('t_mult', b)
                ACTIONS[(b, 3)] = ('t_allred', b)
                ACTIONS[(b, 5)] = ('v_mult', b)
                ACTIONS[(b, 7)] = ('v_red', b)
            ACTIONS[(7, 0)] = ('t_mult', 7)
            ACTIONS[(7, 1)] = ('t_allred', 7)
            ACTIONS[(7, 2)] = ('v_mult', 7)
            ACTIONS[(7, 3)] = ('v_red', 7)
            ACTIONS[(7, 4)] = ('const', 7)

            def do_matmuls(py_cur, c, covr_t, m1_t, m2_t, m3_t):
                xsl = slice(c * 128, (c + 1) * 128)

                def wap(w):
                    return w[:, :].rearrange(
                        "d (two b) -> d two b", two=2)[:, :, xsl]

                def map_(m, s):
                    return m[:, :].rearrange(
                        "d (two f) -> d two f", two=2)[:, :, s:s + 512]

                for m in range(BLK // 512):
                    s = m * 512
                    nc.tensor.matmul(
                        py_cur[:, s:s + 512], xtr[:, xsl],
                        covr_t[:, s:s + 512],
                        start=True, stop=False, skip_group_check=True)
                for m in range(BLK // 512):
                    s = m * 512
                    nc.tensor.matmul(
                        py_cur[:, s:s + 512], wap(w1), map_(m1_t, s),
                        start=False, stop=False, skip_group_check=True,
                        perf_mode=PM.DoubleRow)
                for m in range(BLK // 512):
                    s = m * 512
                    nc.tensor.matmul(
                        py_cur[:, s:s + 512], wap(w2), map_(m2_t, s),
                        start=False, stop=False, skip_group_check=True,
                        perf_mode=PM.DoubleRow)
                for m in range(BLK // 512):
                    s = m * 512
                    nc.tensor.matmul(
                        py_cur[:, s:s + 512], wap(w3), map_(m3_t, s),
                        start=False, stop=(m == BLK // 512 - 1),
                        skip_group_check=True,
                        perf_mode=PM.DoubleRow)

            # ---------- phase 1: blocks outer, chunks inner ----------
            t1a_all = []
            for c in range(N_CHUNKS):
                t1a_c = t1ap.tile([128, K], FP32, tag=f"t1a{c}")
                t1a_all.append(t1a_c)

            prepped = prep_block(1)

            for blk in range(N_BLK):
                if blk == 0:
                    covr_t, m1_t, m2_t, m3_t, covf_t, covt_t = prep0
                else:
                    covr_t, m1_t, m2_t, m3_t, covf_t, covt_t = prepped
                    if blk + 1 < N_BLK:
                        prepped = prep_block(blk + 1)

                for c in range(N_CHUNKS):
                    py_cur = pyp.tile([128, BLK], FP32, tag="py")
                    do_matmuls(py_cur, c, covr_t, m1_t, m2_t, m3_t)

                    nf = NF_CHUNK[c]
                    nred = NGRP - nf
                    t1a = t1a_all[c]
                    w = nred * 128
                    for f in range(nf):
                        g = nred + f
                        sc = workp.tile([128, 128], FP32, tag="sqscratch")
                        nc.scalar.activation(
                            sc[:, :], py_cur[:, g * 128:(g + 1) * 128],
                            AF.Square, scale=SQRT_A,
                            accum_out=t1a[:, blk * NGRP + g:
                                          blk * NGRP + g + 1])
                    ysq = ysqp.tile([128, 16 * 128], FP32, tag="ysq")
                    nc.scalar.activation(ysq[:, 0:w], py_cur[:, 0:w],
                                         AF.Square, scale=SQRT_A)
                    nc.vector.tensor_reduce(
                        out=t1a[:, blk * NGRP:blk * NGRP + nred],
                        in_=ysq[:, 0:w].rearrange("b (g j) -> b g j", j=128),
                        axis=AX.X, op=OP.add)

                    key = (blk, c)
                    if key in ACTIONS:
                        act, b = ACTIONS[key]
                        if act == 't_mult':
                            t_mult(b, covf_t)
                        elif act == 't_allred':
                            t_allred(b)
                        elif act == 'v_mult':
                            v_mult(b, covt_t)
                        elif act == 'v_red':
                            v_red(b)
                        elif act == 'const':
                            const_chain()

            # ---------- phase 2 epilogue: logits + softmax per chunk ----
            for c in range(N_CHUNKS):
                t1a = t1a_all[c]
                lhsT = xtf[:, c * 128:(c + 1) * 128]
                pl = pyp.tile([128, K], FP32, tag="py")
                nc.tensor.matmul(pl[:, :], lhsT, vt2a_sb[:, :],
                                 start=True, stop=False)
                nc.tensor.matmul(pl[:, :], ones_row[:, :],
                                 const_row[:, :],
                                 start=False, stop=True)
                lg = workp.tile([128, K], FP32, tag="lg")
                nc.vector.tensor_tensor(out=lg[:, :], in0=pl[:, :],
                                        in1=t1a[:, :], op=OP.subtract)
                nmx = workp.tile([128, 1], FP32, tag="nmx")
                nc.vector.tensor_reduce(out=nmx[:, :], in_=lg[:, :],
                                        axis=AX.X, op=OP.max, negate=True)
                ex = workp.tile([128, K], FP32, tag="ex")
                den = workp.tile([128, 1], FP32, tag="den")
                nc.scalar.activation(ex[:, :], lg[:, :], AF.Exp,
                                     bias=nmx[:, 0:1],
                                     accum_out=den[:, 0:1])
                rden = workp.tile([128, 1], FP32, tag="rden")
                nc.vector.reciprocal(rden[:, :], den[:, :])
                ot = workp.tile([128, K], FP32, tag="ot")
                nc.gpsimd.tensor_scalar(out=ot[:, :], in0=ex[:, :],
                                        scalar1=rden[:, 0:1],
                                        scalar2=None, op0=OP.mult)
                nc.sync.dma_start(
                    out=out_d[c * 128:(c + 1) * 128, :],
                    in_=ot[:, :])

    nc.compile()
    return nc


_NC_CACHE = None


def _rn12(a):
    u = np.ascontiguousarray(a, dtype=np.float32).view(np.uint32)
    add = np.uint32(0x800) - ((u >> np.uint32(12)) & np.uint32(1))
    u2 = (u + add) & np.uint32(0xFFFFF000)
    return u2.view(np.float32)


def kernel(x, centers, cov, mixers):
    global _NC_CACHE
    import ml_dtypes
    from concourse.bass_utils import run_bass_kernel_spmd

    E5np = ml_dtypes.float8_e5m2
    E4np = ml_dtypes.float8_e4m3fn

    if _NC_CACHE is None:
        _NC_CACHE = _build_bass()
    nc = _NC_CACHE

    x = np.ascontiguousarray(x, dtype=np.float32)
    cov2 = np.ascontiguousarray(cov, dtype=np.float32).reshape(K * D, D)
    cen = np.ascontiguousarray(centers, dtype=np.float32)
    mix = np.ascontiguousarray(mixers, dtype=np.float32)

    def e5(a):
        return a.astype(np.float32).astype(E5np)

    def e4(a):
        return a.astype(np.float32).astype(E4np)

    # ---- c-side packing (shared across cores) ----
    covr = _rn12(cov2)
    Ec = cov2 - covr
    Cf = cov2.astype(np.float64)
    c1 = e5(cov2).astype(np.float64)
    c2 = (e5(((Cf - c1) * 8.0).astype(np.float32)).astype(np.float64) / 8.0)
    ecq = e4(Ec * 1024.0).astype(np.float64) / 1024.0
    ec2 = (Ec.astype(np.float64) - ecq).astype(np.float32)

    # m-tiles in [d, (blk, two, (g, j))] layout, fp8 storage values
    covt = np.ascontiguousarray(
        cov2.reshape(K, D, D).transpose(2, 0, 1).reshape(D, K * D))

    def to_dblk(a):
        # [ (k,d), j ] -> [d, blk, g, j]
        return np.ascontiguousarray(
            a.reshape(N_BLK, NGRP, D, 128).transpose(2, 0, 1, 3))

    m1s0 = e5((c1 / 1024.0).astype(np.float32))         # c1 * 2^-10
    m1s1 = e5((c2 / 1024.0).astype(np.float32))         # c2 * 2^-10
    m2s = e4(Ec * 1024.0)                               # Ec * 2^10
    m3s0 = e5((c1 / 16384.0).astype(np.float32))        # c1 * 2^-14
    m3s1 = e5(ec2 * 16384.0)                            # ec2 * 2^14

    def pack_m(s0, s1):
        a = np.stack([to_dblk(s0), to_dblk(s1)], axis=2)  # [d, blk, 2, g, j]
        return np.ascontiguousarray(a.reshape(D, N_BLK * 2 * BLK))

    m1 = pack_m(m1s0, m1s1)
    m2 = pack_m(m2s, m2s)
    m3 = pack_m(m3s0, m3s1)

    cent2a = np.ascontiguousarray((2.0 * ALPHA) * cen.T)
    cent = np.ascontiguousarray(cen.T)

    in_maps = []
    for c in range(N_CORES):
        xs = x[c * B_LOC:(c + 1) * B_LOC]
        xt = np.ascontiguousarray(xs.T)             # [D, B_LOC]
        xtr = _rn12(xt)
        Ex = xt - xtr
        exq = e4(Ex * 1024.0).astype(np.float64) / 1024.0
        ex2 = (Ex.astype(np.float64) - exq).astype(np.float32)
        x1 = e5(xt).astype(np.float64)
        x2 = e5(((xt.astype(np.float64) - x1) * 8.0).astype(np.float32))

        w1s = e4(Ex * 1024.0)                            # Ex * 2^10
        w2s0 = e5((x1 / 1024.0).astype(np.float32))      # x1 * 2^-10
        w2s1 = (x2.astype(np.float64) / 8.0 / 1024.0)    # x2 * 2^-10
        w2s1 = e5(w2s1.astype(np.float32))
        w3s0 = e5(ex2 * 16384.0)                         # ex2 * 2^14
        w3s1 = e5(xt / 16384.0)                          # x * 2^-14

        def pack_w(s0, s1):
            a = np.stack([s0, s1], axis=1)               # [d, 2, b]
            return np.ascontiguousarray(a.reshape(D, 2 * B_LOC))

        in_maps.append({
            "xtr": xtr,
            "xtf": xt,
            "w1": pack_w(w1s, w1s),
            "w2": pack_w(w2s0, w2s1),
            "w3": pack_w(w3s0, w3s1),
            "covr": covr,
            "covf": cov2,
            "covt": covt,
            "m1": m1,
            "m2": m2,
            "m3": m3,
            "cent2a": cent2a,
            "cent": cent,
            "mixers": mix,
        })
    res = run_bass_kernel_spmd(nc, in_maps, list(range(N_CORES)))
    out = np.concatenate([res.results[c]["out"] for c in range(N_CORES)],
                         axis=0)
    return out
